# revision 25
# baseline (speedup 1.0000x reference)
"""Multi-head attention (B=2, S=2048, EMB=1024, 16 heads) on 8 Trainium2 cores.

Sharding: core c -> batch c//4, head-group c%4 (4 heads = 256 projection dims).
Each core computes its head group's Q/K/V projections in transposed layout
(Q^T/K^T with head-dim on partitions; V natural with a ones-column appended so
the softmax denominator falls out of the ctx matmul), attention without max
subtraction (scores ~ N(0,1); exp can't overflow), and a row-parallel partial
of the output projection.  The host sums the 4 partials per batch and adds the
output bias (the all-reduce of the row-parallel fc_out happens at unshard
time; no device collectives).

All matmul operands are bf16 (PSUM accumulation stays fp32), which halves DMA
traffic and SBUF footprint at ~5e-3 relative error (tolerance 2e-2).  Attention
ctx matmuls are full K=128 contraction chains (one matmul per 128-key tile per
head).  Scores for a head pair land in one two-bank PSUM tile so each exp
covers [128, 1024] in a single ACT instruction.  Inputs arrive as one DMA per
(tensor, chunk) — the hardware DGE serializes DMA issue, so fewer/bigger
transfers matter.  K/V projections are interleaved with the first query
chunk's attention (scores over key-chunk j only need K/V chunk j), and the
normalize / output-projection / next-Q-projection matmuls are pumped one PE
instruction at a time into the ACT-bound attention loop so the tensor engine
never starves the scalar engine.  Q/K biases ride the PSUM->SBUF drain as
per-partition tensor_scalar adds.
"""

import numpy as np
import ml_dtypes

import concourse.tile as tile
from concourse import bacc, mybir
from concourse import bass_utils

EMB = 1024
S = 2048
B = 2
HPC = 4            # heads per core
DQ = HPC * 64      # 256 projection dims per core
NCORES = 8

BF16 = mybir.dt.bfloat16
F32 = mybir.dt.float32
EXP = mybir.ActivationFunctionType.Exp
BF16_NP = ml_dtypes.bfloat16

KT_E = EMB // 128  # 8 contraction tiles over EMB
NQC = S // 512     # 4 query chunks
NST = S // 128     # 16 sequence (key) tiles

_NC = None
TRACE = False
LAST_RESULT = None


def _mha(ctx, tc, xqT, xkT, xvT, wqT, wkT, wvT, woT, bq, bk, bv, out, bench_iters=None):
    nc = tc.nc

    cpool = ctx.enter_context(tc.tile_pool(name="const", bufs=1))
    xpool = ctx.enter_context(tc.tile_pool(name="xkv", bufs=4))
    xqpool = ctx.enter_context(tc.tile_pool(name="xq", bufs=4))
    epool = ctx.enter_context(tc.tile_pool(name="exp", bufs=5))
    upool = ctx.enter_context(tc.tile_pool(name="unorm", bufs=8))
    rpool = ctx.enter_context(tc.tile_pool(name="rec", bufs=4))
    bpool = ctx.enter_context(tc.tile_pool(name="brec", bufs=4))
    opool = ctx.enter_context(tc.tile_pool(name="osb", bufs=3))
    sc_ps = ctx.enter_context(tc.tile_pool(name="scps", bufs=2, space="PSUM"))
    ctx_ps = ctx.enter_context(tc.tile_pool(name="ctxps", bufs=2, space="PSUM"))
    mm_ps = ctx.enter_context(tc.tile_pool(name="mmps", bufs=2, space="PSUM"))

    # ---- persistent SBUF tensors ----
    wk_sb = cpool.tile([128, KT_E * DQ], BF16)  # wk_sb[p, kt*256+m] = WkT[kt*128+p, m]
    wv_sb = cpool.tile([128, KT_E * DQ], BF16)
    wq_sb = cpool.tile([128, KT_E * DQ], BF16)
    bk2 = cpool.tile([128, 2], F32)             # bk2[p, d] = bk[d*128+p]
    bq2 = cpool.tile([128, 2], F32)
    bv_sb = cpool.tile([1, DQ], F32)
    vb128 = cpool.tile([128, DQ], F32)          # bv broadcast to all partitions
    wo_sb = cpool.tile([128, 2 * EMB], BF16)    # wo_sb[p, n*1024+f] = WoT[n*128+p, f]
    def dma_w():
        nc.sync.dma_start(
            wk_sb[:].rearrange("p (n m) -> p n m", n=KT_E),
            wkT.rearrange("(n p) m -> p n m", p=128),
        )
        yield
        nc.sync.dma_start(
            wv_sb[:].rearrange("p (n m) -> p n m", n=KT_E),
            wvT.rearrange("(n p) m -> p n m", p=128),
        )
        yield
        nc.sync.dma_start(
            wq_sb[:].rearrange("p (n m) -> p n m", n=KT_E),
            wqT.rearrange("(n p) m -> p n m", p=128),
        )
        yield
        for sb, src in ((bk2, bk), (bq2, bq)):
            nc.sync.dma_start(sb[:], src.rearrange("o (d p) -> p (o d)", p=128))
        nc.sync.dma_start(bv_sb[:], bv[:])
        nc.gpsimd.partition_broadcast(vb128[:], bv_sb[:], channels=128)
        nc.sync.dma_start(
            wo_sb[:].rearrange("p (n m) -> p n m", n=2),
            woT.rearrange("(n p) m -> p n m", p=128),
        )
        yield

    # results of projections kept resident
    kT_sb = cpool.tile([128, 2 * S], BF16)      # [dq-block 2][s 2048]
    qT_sb = cpool.tile([128, 2 * S], BF16)
    ctxT_sb = cpool.tile([128, 2 * S], BF16)
    v_sb = cpool.tile([128, NST * (HPC * 65)], BF16)  # per s-tile: 4 heads x (64 V + ones)
    nc.vector.memset(
        v_sb[:].rearrange("p (t h m) -> p t h m", t=NST, h=HPC)[:, :, :, 64:65],
        1.0,
    )

    for _ in dma_w():
        pass

    def body():
        _body(tc, nc, xqT, xkT, xvT, out, vb128, wq_sb, wk_sb,
              wv_sb, wo_sb, bq2, bk2, kT_sb, qT_sb, ctxT_sb, v_sb,
              xpool, xqpool, epool, upool, rpool, bpool, opool,
              sc_ps, ctx_ps, mm_ps)

    if bench_iters:
        hints = (
            mybir.EngineType.PE,
            mybir.EngineType.Activation,
            mybir.EngineType.DVE,
            mybir.EngineType.SP,
            mybir.EngineType.Pool,
        )
        with tc.For_i(0, bench_iters, 1, hint_engines=hints):
            body()
    else:
        body()


def _body(tc, nc, xqT, xkT, xvT, out, vb128, wq_sb, wk_sb,
          wv_sb, wo_sb, bq2, bk2, kT_sb, qT_sb, ctxT_sb, v_sb,
          xpool, xqpool, epool, upool, rpool, bpool, opool,
          sc_ps, ctx_ps, mm_ps):

    # ---- input DMAs: one per (tensor, chunk); DGE issue is the scarce
    # resource, transfer itself spreads over all DMA engines ----
    xk_all, xv_all, xq_all = {}, {}, {}
    def dma_x(store, pool, src, c, nm):
        t = pool.tile([128, KT_E * 512], BF16, tag=nm[:2], name=f"{nm}_{c}")
        half = KT_E // 4
        for h in range(4):
            nc.sync.dma_start(
                t[:, h * half * 512:(h + 1) * half * 512].rearrange(
                    "p (n m) -> p n m", n=half),
                src[h * half:(h + 1) * half, c].rearrange("n p m -> p n m"),
            )
        store[c] = t

    dma_x(xk_all, xpool, xkT, 0, "xk")
    dma_x(xv_all, xpool, xvT, 0, "xv")
    dma_x(xq_all, xqpool, xqT, 0, "xq")
    for c in range(1, NQC):
        dma_x(xk_all, xpool, xkT, c, "xk")
        dma_x(xv_all, xpool, xvT, c, "xv")
        dma_x(xq_all, xqpool, xqT, c, "xq")

    def kv_proj(c):
        xk, xv = xk_all[c], xv_all[c]
        for dq in range(2):
            ps = mm_ps.tile([128, 512], F32, tag="mm", name=f"kps_{c}_{dq}")
            col = lambda kt: kt * DQ + dq * 128
            for kt in range(KT_E):
                nc.tensor.matmul(ps[:], wk_sb[:, col(kt): col(kt) + 128],
                                 xk[:, kt * 512: kt * 512 + 512],
                                 start=(kt == 0), stop=(kt == KT_E - 1))
            nc.vector.tensor_scalar_add(
                kT_sb[:, dq * S + c * 512: dq * S + c * 512 + 512], ps[:],
                bk2[:, dq: dq + 1])
        for sti in range(4):
            st = c * 4 + sti
            ps = mm_ps.tile([128, 256], F32, tag="mm", name=f"vps_{st}")
            for kt in range(KT_E):
                nc.tensor.matmul(ps[:], xv[:, kt * 512 + sti * 128: kt * 512 + sti * 128 + 128],
                                 wv_sb[:, kt * DQ: kt * DQ + DQ],
                                 start=(kt == 0), stop=(kt == KT_E - 1))
            dst = v_sb[:, st * (HPC * 65): (st + 1) * (HPC * 65)]
            nc.vector.tensor_add(
                dst.rearrange("p (h m) -> p h m", h=HPC)[:, :, 0:64],
                ps[:].rearrange("p (h m) -> p h m", h=HPC),
                vb128[:].rearrange("p (h m) -> p h m", h=HPC),
            )

    # ---- PE filler generators (pumped into the attention loop) ----
    def gen_qproj(qc):
        xq = xq_all[qc]
        for dq in range(2):
            ps = mm_ps.tile([128, 512], F32, tag="mm", name=f"qps_{qc}_{dq}")
            col = lambda kt: kt * DQ + dq * 128
            for kt in range(KT_E):
                nc.tensor.matmul(ps[:], wq_sb[:, col(kt): col(kt) + 128],
                                 xq[:, kt * 512: kt * 512 + 512],
                                 start=(kt == 0), stop=(kt == KT_E - 1))
                yield
            nc.vector.tensor_scalar_add(
                qT_sb[:, dq * S + qc * 512: dq * S + qc * 512 + 512], ps[:],
                bq2[:, dq: dq + 1])
            yield

    pending_norm = {}

    def do_norm(qc, hp):
        for hi in range(2):
            u = pending_norm.pop((qc, hp, hi))
            rec = rpool.tile([1, 512], F32, tag="rec", name=f"rec_{qc}_{hp}_{hi}")
            nc.vector.reciprocal(rec[:], u[64:65, :])
            brec = bpool.tile([64, 512], F32, tag="br", name=f"br_{qc}_{hp}_{hi}")
            nc.gpsimd.partition_broadcast(brec[:], rec[:], channels=64)
            nc.vector.tensor_mul(
                ctxT_sb[64 * hi: 64 * hi + 64, hp * S + qc * 512: hp * S + qc * 512 + 512],
                u[0:64, :], brec[:],
            )

    def gen_outproj(qc):
        last = qc == NQC - 1
        for qt4 in range(4):
            qt = qc * 4 + qt4
            ot = opool.tile([128, EMB], BF16, tag="o", name=f"ot_{qt}")
            for fc in range(2):
                ps = mm_ps.tile([128, 512], F32, tag="mm", name=f"ops_{qt}_{fc}")
                nc.tensor.matmul(ps[:], ctxT_sb[:, qt * 128: qt * 128 + 128],
                                 wo_sb[:, fc * 512: fc * 512 + 512],
                                 start=True, stop=False)
                yield
                nc.tensor.matmul(ps[:], ctxT_sb[:, S + qt * 128: S + qt * 128 + 128],
                                 wo_sb[:, EMB + fc * 512: EMB + fc * 512 + 512],
                                 start=False, stop=True)
                yield
                # final chunk: ACT is idle by now, DVE is the tail's critical
                # path — drain via the scalar engine instead
                if last:
                    nc.scalar.copy(ot[:, fc * 512: fc * 512 + 512], ps[:])
                else:
                    nc.vector.tensor_copy(ot[:, fc * 512: fc * 512 + 512], ps[:])
            # alternate DGE paths (SWDGE via Pool, HWDGE via SP) so the last
            # few output transfers issue in parallel
            if qt4 % 2 == 0:
                nc.gpsimd.dma_start(out[qt * 128:(qt + 1) * 128, :], ot[:])
            else:
                nc.sync.dma_start(out[qt * 128:(qt + 1) * 128, :], ot[:])

    fillers = []

    def pump(n=1):
        for _ in range(n):
            while fillers:
                try:
                    next(fillers[0])
                    return_ = True
                    break
                except StopIteration:
                    fillers.pop(0)
            else:
                return

    # deferred ctx queue (depth 2): each ctx lands two scores/exp steps after
    # its own, across block boundaries — the exp->ctx dependency gets ~2 kt of
    # slack to absorb ACT latency, and block tails never break the pipeline.
    pending = []

    def flush_one():
        if not pending:
            return
        qc, hp, cps, e, kt = pending.pop(0)
        ctx_mms(qc, hp, cps, e, kt)
        if kt == NST - 1:
            attn_drain(qc, hp, cps)

    def attn_block(qc, hp, cps, kts):
        for kt in kts:
            sc = sc_ps.tile([128, 1024], F32, tag="sc", name=f"sc_{qc}_{hp}_{kt}")
            for hi in range(2):
                nc.tensor.matmul(
                    sc[:, hi * 512: hi * 512 + 512],
                    kT_sb[64 * hi: 64 * hi + 64,
                          hp * S + kt * 128: hp * S + kt * 128 + 128],
                    qT_sb[64 * hi: 64 * hi + 64,
                          hp * S + qc * 512: hp * S + qc * 512 + 512],
                    start=True, stop=True,
                )
            e = epool.tile([128, 1024], BF16, tag="e", name=f"e_{qc}_{hp}_{kt}")
            nc.scalar.activation(e[:], sc[:], EXP, scale=0.125)
            if len(pending) >= 2:
                flush_one()
            pending.append((qc, hp, cps, e, kt))
            pump(1)

    def ctx_mms(qc, hp, cps, e, kt):
        for hi in range(2):
            vcol = kt * (HPC * 65) + (hp * 2 + hi) * 65
            nc.tensor.matmul(
                cps[hi][:], v_sb[:, vcol: vcol + 65],
                e[:, hi * 512: hi * 512 + 512],
                start=(kt == 0), stop=(kt == NST - 1),
            )

    def attn_drain(qc, hp, cps):
        for hi in range(2):
            u = upool.tile([65, 512], F32, tag="u", name=f"u_{qc}_{hp}_{hi}")
            if qc == NQC - 1 and hp == 1:
                nc.scalar.copy(u[:], cps[hi][:])
            else:
                nc.vector.tensor_copy(u[:], cps[hi][:])
            pending_norm[(qc, hp, hi)] = u
        do_norm(qc, hp)
        if hp == 1:
            fillers.append(gen_outproj(qc))

    def new_cps(qc, hp):
        return [
            ctx_ps.tile([65, 512], F32, tag="ctx", name=f"ctx_{qc}_{hp}_{hi}")
            for hi in range(2)
        ]

    # ---- phase order: KV(c0) -> Q(0) -> qc0-hp0 in 4-kt blocks staggered
    # with KV(c1..3) -> qc0-hp1 -> qc1..3 with pumped fillers ----
    kv_proj(0)
    for _ in gen_qproj(0):
        pass

    cps = new_cps(0, 0)
    for c in range(1, NQC + 1):
        attn_block(0, 0, cps, range((c - 1) * 4, c * 4))
        if c < NQC:
            kv_proj(c)

    fillers.append(gen_qproj(1))
    cps = new_cps(0, 1)
    attn_block(0, 1, cps, range(NST))

    for qc in range(1, NQC):
        if qc + 1 < NQC:
            fillers.append(gen_qproj(qc + 1))
        for hp in range(2):
            cps = new_cps(qc, hp)
            attn_block(qc, hp, cps, range(NST))
    while pending:
        flush_one()

    while fillers:
        try:
            next(fillers[0])
        except StopIteration:
            fillers.pop(0)


def _build_nc(bench_iters=None):
    from contextlib import ExitStack

    nc = bacc.Bacc("TRN2", target_bir_lowering=False, debug=False, num_devices=NCORES)
    xqT = nc.dram_tensor("xqT", [KT_E, NQC, 128, 512], BF16, kind="ExternalInput").ap()
    xkT = nc.dram_tensor("xkT", [KT_E, NQC, 128, 512], BF16, kind="ExternalInput").ap()
    xvT = nc.dram_tensor("xvT", [KT_E, NQC, 128, 512], BF16, kind="ExternalInput").ap()
    wqT = nc.dram_tensor("wqT", [EMB, DQ], BF16, kind="ExternalInput").ap()
    wkT = nc.dram_tensor("wkT", [EMB, DQ], BF16, kind="ExternalInput").ap()
    wvT = nc.dram_tensor("wvT", [EMB, DQ], BF16, kind="ExternalInput").ap()
    woT = nc.dram_tensor("woT", [DQ, EMB], BF16, kind="ExternalInput").ap()
    bq = nc.dram_tensor("bq", [1, DQ], F32, kind="ExternalInput").ap()
    bk = nc.dram_tensor("bk", [1, DQ], F32, kind="ExternalInput").ap()
    bv = nc.dram_tensor("bv", [1, DQ], F32, kind="ExternalInput").ap()
    out = nc.dram_tensor("out", [S, EMB], BF16, kind="ExternalOutput").ap()

    with ExitStack() as ctx:
        tc = ctx.enter_context(tile.TileContext(nc))
        _mha(ctx, tc, xqT, xkT, xvT, wqT, wkT, wvT, woT, bq, bk, bv, out,
             bench_iters=bench_iters)
    nc.compile()
    return nc


def _chunk_major(x):
    """[S, EMB] fp32 -> bf16 x.T chunked as [KT_E, NQC, 128, 512] (contiguous)."""
    xt = np.asarray(x, np.float32).T.astype(BF16_NP)  # [EMB, S]
    return np.ascontiguousarray(
        xt.reshape(KT_E, 128, NQC, 512).transpose(0, 2, 1, 3)
    )


def _bf(x):
    return np.ascontiguousarray(np.asarray(x, np.float32).astype(BF16_NP))


def make_in_maps(inputs):
    i = {k: np.asarray(v, np.float32) for k, v in inputs.items()}
    xcm = {}
    for nm in ("query", "key", "value"):
        for b in range(B):
            xcm[(nm, b)] = _chunk_major(i[nm][b])
    in_maps = []
    for c in range(NCORES):
        b, g = divmod(c, 4)
        rows = slice(g * DQ, (g + 1) * DQ)
        in_maps.append({
            "xqT": xcm[("query", b)],
            "xkT": xcm[("key", b)],
            "xvT": xcm[("value", b)],
            "wqT": _bf(i["Wq"][rows].T),
            "wkT": _bf(i["Wk"][rows].T),
            "wvT": _bf(i["Wv"][rows].T),
            "woT": _bf(i["Wo"][:, rows].T),
            "bq": np.ascontiguousarray(i["bq"][rows][None, :]),
            "bk": np.ascontiguousarray(i["bk"][rows][None, :]),
            "bv": np.ascontiguousarray(i["bv"][rows][None, :]),
        })
    return in_maps


def kernel(query, key, value, Wq, bq, Wk, bk, Wv, bv, Wo, bo):
    global _NC, LAST_RESULT
    if _NC is None:
        _NC = _build_nc()

    in_maps = make_in_maps({
        "query": query, "key": key, "value": value,
        "Wq": Wq, "bq": bq, "Wk": Wk, "bk": bk,
        "Wv": Wv, "bv": bv, "Wo": Wo, "bo": bo,
    })

    res = bass_utils.run_bass_kernel_spmd(
        _NC, in_maps, core_ids=list(range(NCORES)), trace=TRACE
    )
    LAST_RESULT = res

    out = np.zeros((B, S, EMB), np.float32)
    for c in range(NCORES):
        out[c // 4] += np.asarray(res.results[c]["out"], np.float32)
    out += np.asarray(bo, np.float32)[None, None, :]
    return out


# revision 26
# speedup vs baseline: 1.0017x; 1.0017x over previous
"""Multi-head attention (B=2, S=2048, EMB=1024, 16 heads) on 8 Trainium2 cores.

Sharding: core c -> batch c//4, head-group c%4 (4 heads = 256 projection dims).
Each core computes its head group's Q/K/V projections in transposed layout
(Q^T/K^T with head-dim on partitions; V natural with a ones-column appended so
the softmax denominator falls out of the ctx matmul), attention without max
subtraction (scores ~ N(0,1); exp can't overflow), and a row-parallel partial
of the output projection.  The host sums the 4 partials per batch and adds the
output bias (the all-reduce of the row-parallel fc_out happens at unshard
time; no device collectives).

All matmul operands are bf16 (PSUM accumulation stays fp32), which halves DMA
traffic and SBUF footprint at ~5e-3 relative error (tolerance 2e-2).  Attention
ctx matmuls are full K=128 contraction chains (one matmul per 128-key tile per
head).  Scores for a head pair land in one two-bank PSUM tile so each exp
covers [128, 1024] in a single ACT instruction.  Inputs arrive as one DMA per
(tensor, chunk) — the hardware DGE serializes DMA issue, so fewer/bigger
transfers matter.  K/V projections are interleaved with the first query
chunk's attention (scores over key-chunk j only need K/V chunk j), and the
normalize / output-projection / next-Q-projection matmuls are pumped one PE
instruction at a time into the ACT-bound attention loop so the tensor engine
never starves the scalar engine.  Q/K biases ride the PSUM->SBUF drain as
per-partition tensor_scalar adds.
"""

import numpy as np
import ml_dtypes

import concourse.tile as tile
from concourse import bacc, mybir
from concourse import bass_utils

EMB = 1024
S = 2048
B = 2
HPC = 4            # heads per core
DQ = HPC * 64      # 256 projection dims per core
NCORES = 8

BF16 = mybir.dt.bfloat16
F32 = mybir.dt.float32
EXP = mybir.ActivationFunctionType.Exp
BF16_NP = ml_dtypes.bfloat16

KT_E = EMB // 128  # 8 contraction tiles over EMB
NQC = S // 512     # 4 query chunks
NST = S // 128     # 16 sequence (key) tiles

_NC = None
TRACE = False
LAST_RESULT = None


def _mha(ctx, tc, xqT, xkT, xvT, wqT, wkT, wvT, woT, bq, bk, bv, out, bench_iters=None):
    nc = tc.nc

    cpool = ctx.enter_context(tc.tile_pool(name="const", bufs=1))
    xpool = ctx.enter_context(tc.tile_pool(name="xkv", bufs=4))
    xqpool = ctx.enter_context(tc.tile_pool(name="xq", bufs=4))
    epool = ctx.enter_context(tc.tile_pool(name="exp", bufs=5))
    upool = ctx.enter_context(tc.tile_pool(name="unorm", bufs=8))
    rpool = ctx.enter_context(tc.tile_pool(name="rec", bufs=4))
    bpool = ctx.enter_context(tc.tile_pool(name="brec", bufs=4))
    opool = ctx.enter_context(tc.tile_pool(name="osb", bufs=3))
    sc_ps = ctx.enter_context(tc.tile_pool(name="scps", bufs=2, space="PSUM"))
    ctx_ps = ctx.enter_context(tc.tile_pool(name="ctxps", bufs=2, space="PSUM"))
    mm_ps = ctx.enter_context(tc.tile_pool(name="mmps", bufs=2, space="PSUM"))

    # ---- persistent SBUF tensors ----
    wk_sb = cpool.tile([128, KT_E * DQ], BF16)  # wk_sb[p, kt*256+m] = WkT[kt*128+p, m]
    wv_sb = cpool.tile([128, KT_E * DQ], BF16)
    wq_sb = cpool.tile([128, KT_E * DQ], BF16)
    bk2 = cpool.tile([128, 2], F32)             # bk2[p, d] = bk[d*128+p]
    bq2 = cpool.tile([128, 2], F32)
    bv_sb = cpool.tile([1, DQ], F32)
    vb128 = cpool.tile([128, DQ], F32)          # bv broadcast to all partitions
    wo_sb = cpool.tile([128, 2 * EMB], BF16)    # wo_sb[p, n*1024+f] = WoT[n*128+p, f]
    def dma_w():
        nc.sync.dma_start(
            wk_sb[:].rearrange("p (n m) -> p n m", n=KT_E),
            wkT.rearrange("(n p) m -> p n m", p=128),
        )
        yield
        nc.sync.dma_start(
            wv_sb[:].rearrange("p (n m) -> p n m", n=KT_E),
            wvT.rearrange("(n p) m -> p n m", p=128),
        )
        yield
        nc.sync.dma_start(
            wq_sb[:].rearrange("p (n m) -> p n m", n=KT_E),
            wqT.rearrange("(n p) m -> p n m", p=128),
        )
        yield
        for sb, src in ((bk2, bk), (bq2, bq)):
            nc.sync.dma_start(sb[:], src.rearrange("o (d p) -> p (o d)", p=128))
        nc.sync.dma_start(bv_sb[:], bv[:])
        nc.gpsimd.partition_broadcast(vb128[:], bv_sb[:], channels=128)
        nc.sync.dma_start(
            wo_sb[:].rearrange("p (n m) -> p n m", n=2),
            woT.rearrange("(n p) m -> p n m", p=128),
        )
        yield

    # results of projections kept resident
    kT_sb = cpool.tile([128, 2 * S], BF16)      # [dq-block 2][s 2048]
    qT_sb = cpool.tile([128, 2 * S], BF16)
    ctxT_sb = cpool.tile([128, 2 * S], BF16)
    v_sb = cpool.tile([128, NST * (HPC * 65)], BF16)  # per s-tile: 4 heads x (64 V + ones)
    nc.vector.memset(
        v_sb[:].rearrange("p (t h m) -> p t h m", t=NST, h=HPC)[:, :, :, 64:65],
        1.0,
    )

    for _ in dma_w():
        pass

    def body():
        _body(tc, nc, xqT, xkT, xvT, out, vb128, wq_sb, wk_sb,
              wv_sb, wo_sb, bq2, bk2, kT_sb, qT_sb, ctxT_sb, v_sb,
              xpool, xqpool, epool, upool, rpool, bpool, opool,
              sc_ps, ctx_ps, mm_ps)

    if bench_iters:
        hints = (
            mybir.EngineType.PE,
            mybir.EngineType.Activation,
            mybir.EngineType.DVE,
            mybir.EngineType.SP,
            mybir.EngineType.Pool,
        )
        with tc.For_i(0, bench_iters, 1, hint_engines=hints):
            body()
    else:
        body()


def _body(tc, nc, xqT, xkT, xvT, out, vb128, wq_sb, wk_sb,
          wv_sb, wo_sb, bq2, bk2, kT_sb, qT_sb, ctxT_sb, v_sb,
          xpool, xqpool, epool, upool, rpool, bpool, opool,
          sc_ps, ctx_ps, mm_ps):

    # ---- input DMAs: one per (tensor, chunk); DGE issue is the scarce
    # resource, transfer itself spreads over all DMA engines ----
    xk_all, xv_all, xq_all = {}, {}, {}
    def dma_x(store, pool, src, c, nm):
        t = pool.tile([128, KT_E * 512], BF16, tag=nm[:2], name=f"{nm}_{c}")
        half = KT_E // 4
        for h in range(4):
            nc.sync.dma_start(
                t[:, h * half * 512:(h + 1) * half * 512].rearrange(
                    "p (n m) -> p n m", n=half),
                src[h * half:(h + 1) * half, c].rearrange("n p m -> p n m"),
            )
        store[c] = t

    dma_x(xk_all, xpool, xkT, 0, "xk")
    dma_x(xv_all, xpool, xvT, 0, "xv")
    dma_x(xq_all, xqpool, xqT, 0, "xq")
    for c in range(1, NQC):
        dma_x(xk_all, xpool, xkT, c, "xk")
        dma_x(xv_all, xpool, xvT, c, "xv")
        dma_x(xq_all, xqpool, xqT, c, "xq")

    def kv_proj(c):
        xk, xv = xk_all[c], xv_all[c]
        for dq in range(2):
            ps = mm_ps.tile([128, 512], F32, tag="mm", name=f"kps_{c}_{dq}")
            col = lambda kt: kt * DQ + dq * 128
            for kt in range(KT_E):
                nc.tensor.matmul(ps[:], wk_sb[:, col(kt): col(kt) + 128],
                                 xk[:, kt * 512: kt * 512 + 512],
                                 start=(kt == 0), stop=(kt == KT_E - 1))
            nc.vector.tensor_scalar_add(
                kT_sb[:, dq * S + c * 512: dq * S + c * 512 + 512], ps[:],
                bk2[:, dq: dq + 1])
        for sti in range(4):
            st = c * 4 + sti
            ps = mm_ps.tile([128, 256], F32, tag="mm", name=f"vps_{st}")
            for kt in range(KT_E):
                nc.tensor.matmul(ps[:], xv[:, kt * 512 + sti * 128: kt * 512 + sti * 128 + 128],
                                 wv_sb[:, kt * DQ: kt * DQ + DQ],
                                 start=(kt == 0), stop=(kt == KT_E - 1))
            dst = v_sb[:, st * (HPC * 65): (st + 1) * (HPC * 65)]
            nc.vector.tensor_add(
                dst.rearrange("p (h m) -> p h m", h=HPC)[:, :, 0:64],
                ps[:].rearrange("p (h m) -> p h m", h=HPC),
                vb128[:].rearrange("p (h m) -> p h m", h=HPC),
            )

    # ---- PE filler generators (pumped into the attention loop) ----
    def gen_qproj(qc):
        xq = xq_all[qc]
        for dq in range(2):
            ps = mm_ps.tile([128, 512], F32, tag="mm", name=f"qps_{qc}_{dq}")
            col = lambda kt: kt * DQ + dq * 128
            for kt in range(KT_E):
                nc.tensor.matmul(ps[:], wq_sb[:, col(kt): col(kt) + 128],
                                 xq[:, kt * 512: kt * 512 + 512],
                                 start=(kt == 0), stop=(kt == KT_E - 1))
                yield
            nc.vector.tensor_scalar_add(
                qT_sb[:, dq * S + qc * 512: dq * S + qc * 512 + 512], ps[:],
                bq2[:, dq: dq + 1])
            yield

    pending_norm = {}

    def do_norm(qc, hp):
        for hi in range(2):
            u = pending_norm.pop((qc, hp, hi))
            rec = rpool.tile([1, 512], F32, tag="rec", name=f"rec_{qc}_{hp}_{hi}")
            nc.vector.reciprocal(rec[:], u[64:65, :])
            brec = bpool.tile([64, 512], F32, tag="br", name=f"br_{qc}_{hp}_{hi}")
            nc.gpsimd.partition_broadcast(brec[:], rec[:], channels=64)
            nc.vector.tensor_mul(
                ctxT_sb[64 * hi: 64 * hi + 64, hp * S + qc * 512: hp * S + qc * 512 + 512],
                u[0:64, :], brec[:],
            )

    def gen_outproj(qc):
        last = qc == NQC - 1
        for qt4 in range(4):
            qt = qc * 4 + qt4
            ot = opool.tile([128, EMB], BF16, tag="o", name=f"ot_{qt}")
            for fc in range(2):
                ps = mm_ps.tile([128, 512], F32, tag="mm", name=f"ops_{qt}_{fc}")
                nc.tensor.matmul(ps[:], ctxT_sb[:, qt * 128: qt * 128 + 128],
                                 wo_sb[:, fc * 512: fc * 512 + 512],
                                 start=True, stop=False)
                yield
                nc.tensor.matmul(ps[:], ctxT_sb[:, S + qt * 128: S + qt * 128 + 128],
                                 wo_sb[:, EMB + fc * 512: EMB + fc * 512 + 512],
                                 start=False, stop=True)
                yield
                # final chunk: ACT is idle by now, DVE is the tail's critical
                # path — drain via the scalar engine instead
                if last:
                    nc.scalar.copy(ot[:, fc * 512: fc * 512 + 512], ps[:])
                else:
                    nc.vector.tensor_copy(ot[:, fc * 512: fc * 512 + 512], ps[:])
            # alternate DGE paths (SWDGE via Pool, HWDGE via SP) so the last
            # few output transfers issue in parallel
            if qt4 % 2 == 0:
                nc.gpsimd.dma_start(out[qt * 128:(qt + 1) * 128, :], ot[:])
            else:
                nc.sync.dma_start(out[qt * 128:(qt + 1) * 128, :], ot[:])

    fillers = []

    def pump(n=1):
        for _ in range(n):
            while fillers:
                try:
                    next(fillers[0])
                    return_ = True
                    break
                except StopIteration:
                    fillers.pop(0)
            else:
                return

    # deferred ctx queue (depth 2): each ctx lands two scores/exp steps after
    # its own, across block boundaries — the exp->ctx dependency gets ~2 kt of
    # slack to absorb ACT latency, and block tails never break the pipeline.
    pending = []

    def flush_one():
        if not pending:
            return
        qc, hp, cps, e, kt = pending.pop(0)
        ctx_mms(qc, hp, cps, e, kt)
        if kt == NST - 1:
            attn_drain(qc, hp, cps)

    def attn_block(qc, hp, cps, kts):
        for kt in kts:
            sc = sc_ps.tile([128, 1024], F32, tag="sc", name=f"sc_{qc}_{hp}_{kt}")
            for hi in range(2):
                nc.tensor.matmul(
                    sc[:, hi * 512: hi * 512 + 512],
                    kT_sb[64 * hi: 64 * hi + 64,
                          hp * S + kt * 128: hp * S + kt * 128 + 128],
                    qT_sb[64 * hi: 64 * hi + 64,
                          hp * S + qc * 512: hp * S + qc * 512 + 512],
                    start=True, stop=True,
                )
            e = epool.tile([128, 1024], BF16, tag="e", name=f"e_{qc}_{hp}_{kt}")
            nc.scalar.activation(e[:], sc[:], EXP, scale=0.125)
            if len(pending) >= 2:
                flush_one()
            pending.append((qc, hp, cps, e, kt))
            pump(1)

    def ctx_mms(qc, hp, cps, e, kt):
        for hi in range(2):
            vcol = kt * (HPC * 65) + (hp * 2 + hi) * 65
            nc.tensor.matmul(
                cps[hi][:], v_sb[:, vcol: vcol + 65],
                e[:, hi * 512: hi * 512 + 512],
                start=(kt == 0), stop=(kt == NST - 1),
            )

    def attn_drain(qc, hp, cps):
        for hi in range(2):
            u = upool.tile([65, 512], F32, tag="u", name=f"u_{qc}_{hp}_{hi}")
            if qc == NQC - 1 and hp == 1:
                nc.scalar.copy(u[:], cps[hi][:])
            else:
                nc.vector.tensor_copy(u[:], cps[hi][:])
            pending_norm[(qc, hp, hi)] = u
        do_norm(qc, hp)
        if hp == 1:
            fillers.append(gen_outproj(qc))

    def new_cps(qc, hp):
        return [
            ctx_ps.tile([65, 512], F32, tag="ctx", name=f"ctx_{qc}_{hp}_{hi}")
            for hi in range(2)
        ]

    # ---- phase order: KV(c0) -> Q(0) -> qc0-hp0 in 4-kt blocks staggered
    # with KV(c1..3) -> qc0-hp1 -> qc1..3 with pumped fillers ----
    kv_proj(0)
    for _ in gen_qproj(0):
        pass

    cps = new_cps(0, 0)
    for c in range(1, NQC + 1):
        attn_block(0, 0, cps, range((c - 1) * 4, c * 4))
        if c < NQC:
            kv_proj(c)

    fillers.append(gen_qproj(1))
    cps = new_cps(0, 1)
    attn_block(0, 1, cps, range(NST))

    for qc in range(1, NQC):
        if qc + 1 < NQC:
            fillers.append(gen_qproj(qc + 1))
        for hp in range(2):
            cps = new_cps(qc, hp)
            attn_block(qc, hp, cps, range(NST))
    while pending:
        flush_one()

    while fillers:
        try:
            next(fillers[0])
        except StopIteration:
            fillers.pop(0)


def _build_nc(bench_iters=None):
    from contextlib import ExitStack

    nc = bacc.Bacc("TRN2", target_bir_lowering=False, debug=False, num_devices=NCORES,
                   use_seq_codegen=True)
    xqT = nc.dram_tensor("xqT", [KT_E, NQC, 128, 512], BF16, kind="ExternalInput").ap()
    xkT = nc.dram_tensor("xkT", [KT_E, NQC, 128, 512], BF16, kind="ExternalInput").ap()
    xvT = nc.dram_tensor("xvT", [KT_E, NQC, 128, 512], BF16, kind="ExternalInput").ap()
    wqT = nc.dram_tensor("wqT", [EMB, DQ], BF16, kind="ExternalInput").ap()
    wkT = nc.dram_tensor("wkT", [EMB, DQ], BF16, kind="ExternalInput").ap()
    wvT = nc.dram_tensor("wvT", [EMB, DQ], BF16, kind="ExternalInput").ap()
    woT = nc.dram_tensor("woT", [DQ, EMB], BF16, kind="ExternalInput").ap()
    bq = nc.dram_tensor("bq", [1, DQ], F32, kind="ExternalInput").ap()
    bk = nc.dram_tensor("bk", [1, DQ], F32, kind="ExternalInput").ap()
    bv = nc.dram_tensor("bv", [1, DQ], F32, kind="ExternalInput").ap()
    out = nc.dram_tensor("out", [S, EMB], BF16, kind="ExternalOutput").ap()

    with ExitStack() as ctx:
        tc = ctx.enter_context(tile.TileContext(nc))
        _mha(ctx, tc, xqT, xkT, xvT, wqT, wkT, wvT, woT, bq, bk, bv, out,
             bench_iters=bench_iters)
    nc.compile()
    return nc


def _chunk_major(x):
    """[S, EMB] fp32 -> bf16 x.T chunked as [KT_E, NQC, 128, 512] (contiguous)."""
    xt = np.asarray(x, np.float32).T.astype(BF16_NP)  # [EMB, S]
    return np.ascontiguousarray(
        xt.reshape(KT_E, 128, NQC, 512).transpose(0, 2, 1, 3)
    )


def _bf(x):
    return np.ascontiguousarray(np.asarray(x, np.float32).astype(BF16_NP))


def make_in_maps(inputs):
    i = {k: np.asarray(v, np.float32) for k, v in inputs.items()}
    xcm = {}
    for nm in ("query", "key", "value"):
        for b in range(B):
            xcm[(nm, b)] = _chunk_major(i[nm][b])
    in_maps = []
    for c in range(NCORES):
        b, g = divmod(c, 4)
        rows = slice(g * DQ, (g + 1) * DQ)
        in_maps.append({
            "xqT": xcm[("query", b)],
            "xkT": xcm[("key", b)],
            "xvT": xcm[("value", b)],
            "wqT": _bf(i["Wq"][rows].T),
            "wkT": _bf(i["Wk"][rows].T),
            "wvT": _bf(i["Wv"][rows].T),
            "woT": _bf(i["Wo"][:, rows].T),
            "bq": np.ascontiguousarray(i["bq"][rows][None, :]),
            "bk": np.ascontiguousarray(i["bk"][rows][None, :]),
            "bv": np.ascontiguousarray(i["bv"][rows][None, :]),
        })
    return in_maps


def kernel(query, key, value, Wq, bq, Wk, bk, Wv, bv, Wo, bo):
    global _NC, LAST_RESULT
    if _NC is None:
        _NC = _build_nc()

    in_maps = make_in_maps({
        "query": query, "key": key, "value": value,
        "Wq": Wq, "bq": bq, "Wk": Wk, "bk": bk,
        "Wv": Wv, "bv": bv, "Wo": Wo, "bo": bo,
    })

    res = bass_utils.run_bass_kernel_spmd(
        _NC, in_maps, core_ids=list(range(NCORES)), trace=TRACE
    )
    LAST_RESULT = res

    out = np.zeros((B, S, EMB), np.float32)
    for c in range(NCORES):
        out[c // 4] += np.asarray(res.results[c]["out"], np.float32)
    out += np.asarray(bo, np.float32)[None, None, :]
    return out


# revision 27
# speedup vs baseline: 1.0050x; 1.0033x over previous
"""Multi-head attention (B=2, S=2048, EMB=1024, 16 heads) on 8 Trainium2 cores.

Sharding: core c -> batch c//4, head-group c%4 (4 heads = 256 projection dims).
Each core computes its head group's Q/K/V projections in transposed layout
(Q^T/K^T with head-dim on partitions; V natural with a ones-column appended so
the softmax denominator falls out of the ctx matmul), attention without max
subtraction (scores ~ N(0,1); exp can't overflow), and a row-parallel partial
of the output projection.  The host sums the 4 partials per batch and adds the
output bias (the all-reduce of the row-parallel fc_out happens at unshard
time; no device collectives).

All matmul operands are bf16 (PSUM accumulation stays fp32), which halves DMA
traffic and SBUF footprint at ~5e-3 relative error (tolerance 2e-2).  Attention
ctx matmuls are full K=128 contraction chains (one matmul per 128-key tile per
head).  Scores for a head pair land in one two-bank PSUM tile so each exp
covers [128, 1024] in a single ACT instruction.  Inputs arrive as one DMA per
(tensor, chunk) — the hardware DGE serializes DMA issue, so fewer/bigger
transfers matter.  K/V projections are interleaved with the first query
chunk's attention (scores over key-chunk j only need K/V chunk j), and the
normalize / output-projection / next-Q-projection matmuls are pumped one PE
instruction at a time into the ACT-bound attention loop so the tensor engine
never starves the scalar engine.  Q/K biases ride the PSUM->SBUF drain as
per-partition tensor_scalar adds.
"""

import numpy as np
import ml_dtypes

import concourse.tile as tile
from concourse import bacc, mybir
from concourse import bass_utils

EMB = 1024
S = 2048
B = 2
HPC = 4            # heads per core
DQ = HPC * 64      # 256 projection dims per core
NCORES = 8

BF16 = mybir.dt.bfloat16
F32 = mybir.dt.float32
EXP = mybir.ActivationFunctionType.Exp
BF16_NP = ml_dtypes.bfloat16

KT_E = EMB // 128  # 8 contraction tiles over EMB
NQC = S // 512     # 4 query chunks
NST = S // 128     # 16 sequence (key) tiles

_NC = None
TRACE = False
LAST_RESULT = None


def _mha(ctx, tc, xqT, xkT, xvT, wqT, wkT, wvT, woT, bq, bk, bv, out, bench_iters=None):
    nc = tc.nc

    cpool = ctx.enter_context(tc.tile_pool(name="const", bufs=1))
    xpool = ctx.enter_context(tc.tile_pool(name="xkv", bufs=4))
    xqpool = ctx.enter_context(tc.tile_pool(name="xq", bufs=4))
    epool = ctx.enter_context(tc.tile_pool(name="exp", bufs=5))
    upool = ctx.enter_context(tc.tile_pool(name="unorm", bufs=8))
    rpool = ctx.enter_context(tc.tile_pool(name="rec", bufs=4))
    bpool = ctx.enter_context(tc.tile_pool(name="brec", bufs=4))
    opool = ctx.enter_context(tc.tile_pool(name="osb", bufs=3))
    sc_ps = ctx.enter_context(tc.tile_pool(name="scps", bufs=2, space="PSUM"))
    ctx_ps = ctx.enter_context(tc.tile_pool(name="ctxps", bufs=2, space="PSUM"))
    mm_ps = ctx.enter_context(tc.tile_pool(name="mmps", bufs=2, space="PSUM"))

    # ---- persistent SBUF tensors ----
    wk_sb = cpool.tile([128, KT_E * DQ], BF16)  # wk_sb[p, kt*256+m] = WkT[kt*128+p, m]
    wv_sb = cpool.tile([128, KT_E * DQ], BF16)
    wq_sb = cpool.tile([128, KT_E * DQ], BF16)
    bk2 = cpool.tile([128, 2], F32)             # bk2[p, d] = bk[d*128+p]
    bq2 = cpool.tile([128, 2], F32)
    bv_sb = cpool.tile([1, DQ], F32)
    vb128 = cpool.tile([128, DQ], F32)          # bv broadcast to all partitions
    wo_sb = cpool.tile([128, 2 * EMB], BF16)    # wo_sb[p, n*1024+f] = WoT[n*128+p, f]
    def dma_w():
        nc.sync.dma_start(
            wk_sb[:].rearrange("p (n m) -> p n m", n=KT_E),
            wkT.rearrange("(n p) m -> p n m", p=128),
        )
        yield
        nc.sync.dma_start(
            wv_sb[:].rearrange("p (n m) -> p n m", n=KT_E),
            wvT.rearrange("(n p) m -> p n m", p=128),
        )
        yield
        nc.sync.dma_start(
            wq_sb[:].rearrange("p (n m) -> p n m", n=KT_E),
            wqT.rearrange("(n p) m -> p n m", p=128),
        )
        yield
        for sb, src in ((bk2, bk), (bq2, bq)):
            nc.sync.dma_start(sb[:], src.rearrange("o (d p) -> p (o d)", p=128))
        nc.sync.dma_start(bv_sb[:], bv[:])
        nc.gpsimd.partition_broadcast(vb128[:], bv_sb[:], channels=128)
        nc.sync.dma_start(
            wo_sb[:].rearrange("p (n m) -> p n m", n=2),
            woT.rearrange("(n p) m -> p n m", p=128),
        )
        yield

    # results of projections kept resident
    kT_sb = cpool.tile([128, 2 * S], BF16)      # [dq-block 2][s 2048]
    qT_sb = cpool.tile([128, 2 * S], BF16)
    ctxT_sb = cpool.tile([128, 2 * S], BF16)
    v_sb = cpool.tile([128, NST * (HPC * 65)], BF16)  # per s-tile: 4 heads x (64 V + ones)
    nc.vector.memset(
        v_sb[:].rearrange("p (t h m) -> p t h m", t=NST, h=HPC)[:, :, :, 64:65],
        1.0,
    )

    for _ in dma_w():
        pass

    def body():
        _body(tc, nc, xqT, xkT, xvT, out, vb128, wq_sb, wk_sb,
              wv_sb, wo_sb, bq2, bk2, kT_sb, qT_sb, ctxT_sb, v_sb,
              xpool, xqpool, epool, upool, rpool, bpool, opool,
              sc_ps, ctx_ps, mm_ps)

    if bench_iters:
        hints = (
            mybir.EngineType.PE,
            mybir.EngineType.Activation,
            mybir.EngineType.DVE,
            mybir.EngineType.SP,
            mybir.EngineType.Pool,
        )
        with tc.For_i(0, bench_iters, 1, hint_engines=hints):
            body()
    else:
        body()


def _body(tc, nc, xqT, xkT, xvT, out, vb128, wq_sb, wk_sb,
          wv_sb, wo_sb, bq2, bk2, kT_sb, qT_sb, ctxT_sb, v_sb,
          xpool, xqpool, epool, upool, rpool, bpool, opool,
          sc_ps, ctx_ps, mm_ps):

    # ---- input DMAs: one per (tensor, chunk); DGE issue is the scarce
    # resource, transfer itself spreads over all DMA engines ----
    xk_all, xv_all, xq_all = {}, {}, {}
    def dma_x(store, pool, src, c, nm):
        t = pool.tile([128, KT_E * 512], BF16, tag=nm[:2], name=f"{nm}_{c}")
        half = KT_E // 4
        for h in range(4):
            nc.sync.dma_start(
                t[:, h * half * 512:(h + 1) * half * 512].rearrange(
                    "p (n m) -> p n m", n=half),
                src[h * half:(h + 1) * half, c].rearrange("n p m -> p n m"),
            )
        store[c] = t

    dma_x(xk_all, xpool, xkT, 0, "xk")
    dma_x(xv_all, xpool, xvT, 0, "xv")
    dma_x(xq_all, xqpool, xqT, 0, "xq")
    for c in range(1, NQC):
        dma_x(xk_all, xpool, xkT, c, "xk")
        dma_x(xv_all, xpool, xvT, c, "xv")
        dma_x(xq_all, xqpool, xqT, c, "xq")

    def kv_proj(c):
        xk, xv = xk_all[c], xv_all[c]
        for dq in range(2):
            ps = mm_ps.tile([128, 512], F32, tag="mm", name=f"kps_{c}_{dq}")
            col = lambda kt: kt * DQ + dq * 128
            for kt in range(KT_E):
                nc.tensor.matmul(ps[:], wk_sb[:, col(kt): col(kt) + 128],
                                 xk[:, kt * 512: kt * 512 + 512],
                                 start=(kt == 0), stop=(kt == KT_E - 1))
            nc.vector.tensor_scalar_add(
                kT_sb[:, dq * S + c * 512: dq * S + c * 512 + 512], ps[:],
                bk2[:, dq: dq + 1])
        for sti in range(4):
            st = c * 4 + sti
            ps = mm_ps.tile([128, 256], F32, tag="mm", name=f"vps_{st}")
            for kt in range(KT_E):
                nc.tensor.matmul(ps[:], xv[:, kt * 512 + sti * 128: kt * 512 + sti * 128 + 128],
                                 wv_sb[:, kt * DQ: kt * DQ + DQ],
                                 start=(kt == 0), stop=(kt == KT_E - 1))
            dst = v_sb[:, st * (HPC * 65): (st + 1) * (HPC * 65)]
            nc.vector.tensor_add(
                dst.rearrange("p (h m) -> p h m", h=HPC)[:, :, 0:64],
                ps[:].rearrange("p (h m) -> p h m", h=HPC),
                vb128[:].rearrange("p (h m) -> p h m", h=HPC),
            )

    # ---- PE filler generators (pumped into the attention loop) ----
    def gen_qproj(qc):
        xq = xq_all[qc]
        for dq in range(2):
            ps = mm_ps.tile([128, 512], F32, tag="mm", name=f"qps_{qc}_{dq}")
            col = lambda kt: kt * DQ + dq * 128
            for kt in range(KT_E):
                nc.tensor.matmul(ps[:], wq_sb[:, col(kt): col(kt) + 128],
                                 xq[:, kt * 512: kt * 512 + 512],
                                 start=(kt == 0), stop=(kt == KT_E - 1))
                yield
            nc.vector.tensor_scalar_add(
                qT_sb[:, dq * S + qc * 512: dq * S + qc * 512 + 512], ps[:],
                bq2[:, dq: dq + 1])
            yield

    pending_norm = {}

    def do_norm(qc, hp):
        for hi in range(2):
            u = pending_norm.pop((qc, hp, hi))
            rec = rpool.tile([1, 512], F32, tag="rec", name=f"rec_{qc}_{hp}_{hi}")
            nc.vector.reciprocal(rec[:], u[64:65, :])
            brec = bpool.tile([64, 512], F32, tag="br", name=f"br_{qc}_{hp}_{hi}")
            nc.gpsimd.partition_broadcast(brec[:], rec[:], channels=64)
            nc.vector.tensor_mul(
                ctxT_sb[64 * hi: 64 * hi + 64, hp * S + qc * 512: hp * S + qc * 512 + 512],
                u[0:64, :], brec[:],
            )

    def gen_outproj(qc):
        last = qc == NQC - 1
        for qt4 in range(4):
            qt = qc * 4 + qt4
            ot = opool.tile([128, EMB], BF16, tag="o", name=f"ot_{qt}")
            for fc in range(2):
                ps = mm_ps.tile([128, 512], F32, tag="mm", name=f"ops_{qt}_{fc}")
                nc.tensor.matmul(ps[:], ctxT_sb[:, qt * 128: qt * 128 + 128],
                                 wo_sb[:, fc * 512: fc * 512 + 512],
                                 start=True, stop=False)
                yield
                nc.tensor.matmul(ps[:], ctxT_sb[:, S + qt * 128: S + qt * 128 + 128],
                                 wo_sb[:, EMB + fc * 512: EMB + fc * 512 + 512],
                                 start=False, stop=True)
                yield
                # final chunk: ACT is idle by now, DVE is the tail's critical
                # path — drain via the scalar engine instead
                if last:
                    nc.scalar.copy(ot[:, fc * 512: fc * 512 + 512], ps[:])
                else:
                    nc.vector.tensor_copy(ot[:, fc * 512: fc * 512 + 512], ps[:])
            # alternate DGE paths (SWDGE via Pool, HWDGE via SP) so the last
            # few output transfers issue in parallel
            if qt4 % 2 == 0:
                nc.gpsimd.dma_start(out[qt * 128:(qt + 1) * 128, :], ot[:])
            else:
                nc.sync.dma_start(out[qt * 128:(qt + 1) * 128, :], ot[:])

    fillers = []

    def pump(n=1):
        for _ in range(n):
            while fillers:
                try:
                    next(fillers[0])
                    return_ = True
                    break
                except StopIteration:
                    fillers.pop(0)
            else:
                return

    # deferred ctx queue (depth 2): each ctx lands two scores/exp steps after
    # its own, across block boundaries — the exp->ctx dependency gets ~2 kt of
    # slack to absorb ACT latency, and block tails never break the pipeline.
    pending = []

    def flush_one():
        if not pending:
            return
        qc, hp, cps, e, kt = pending.pop(0)
        ctx_mms(qc, hp, cps, e, kt)
        if kt == NST - 1:
            attn_drain(qc, hp, cps)

    def attn_block(qc, hp, cps, kts):
        for kt in kts:
            sc = sc_ps.tile([128, 1024], F32, tag="sc", name=f"sc_{qc}_{hp}_{kt}")
            for hi in range(2):
                nc.tensor.matmul(
                    sc[:, hi * 512: hi * 512 + 512],
                    kT_sb[64 * hi: 64 * hi + 64,
                          hp * S + kt * 128: hp * S + kt * 128 + 128],
                    qT_sb[64 * hi: 64 * hi + 64,
                          hp * S + qc * 512: hp * S + qc * 512 + 512],
                    start=True, stop=True,
                )
            e = epool.tile([128, 1024], BF16, tag="e", name=f"e_{qc}_{hp}_{kt}")
            nc.scalar.activation(e[:], sc[:], EXP, scale=0.125)
            if len(pending) >= 2:
                flush_one()
            pending.append((qc, hp, cps, e, kt))
            pump(1)

    def ctx_mms(qc, hp, cps, e, kt):
        for hi in range(2):
            vcol = kt * (HPC * 65) + (hp * 2 + hi) * 65
            nc.tensor.matmul(
                cps[hi][:], v_sb[:, vcol: vcol + 65],
                e[:, hi * 512: hi * 512 + 512],
                start=(kt == 0), stop=(kt == NST - 1),
            )

    def attn_drain(qc, hp, cps):
        for hi in range(2):
            u = upool.tile([65, 512], F32, tag="u", name=f"u_{qc}_{hp}_{hi}")
            if qc == NQC - 1 and hp == 1:
                nc.scalar.copy(u[:], cps[hi][:])
            else:
                nc.vector.tensor_copy(u[:], cps[hi][:])
            pending_norm[(qc, hp, hi)] = u
        do_norm(qc, hp)
        if hp == 1:
            fillers.append(gen_outproj(qc))

    def new_cps(qc, hp):
        return [
            ctx_ps.tile([65, 512], F32, tag="ctx", name=f"ctx_{qc}_{hp}_{hi}")
            for hi in range(2)
        ]

    # ---- phase order: KV(c0) -> Q(0) -> qc0-hp0 in 4-kt blocks staggered
    # with KV(c1..3) -> qc0-hp1 -> qc1..3 with pumped fillers ----
    kv_proj(0)
    for _ in gen_qproj(0):
        pass

    cps = new_cps(0, 0)
    for c in range(1, NQC + 1):
        attn_block(0, 0, cps, range((c - 1) * 4, c * 4))
        if c < NQC:
            kv_proj(c)

    fillers.append(gen_qproj(1))
    cps = new_cps(0, 1)
    attn_block(0, 1, cps, range(NST))

    for qc in range(1, NQC):
        if qc + 1 < NQC:
            fillers.append(gen_qproj(qc + 1))
        for hp in range(2):
            cps = new_cps(qc, hp)
            attn_block(qc, hp, cps, range(NST))
    while pending:
        flush_one()

    while fillers:
        try:
            next(fillers[0])
        except StopIteration:
            fillers.pop(0)


def _build_nc(bench_iters=None):
    from contextlib import ExitStack

    nc = bacc.Bacc("TRN2", target_bir_lowering=False, debug=False, num_devices=NCORES)
    xqT = nc.dram_tensor("xqT", [KT_E, NQC, 128, 512], BF16, kind="ExternalInput").ap()
    xkT = nc.dram_tensor("xkT", [KT_E, NQC, 128, 512], BF16, kind="ExternalInput").ap()
    xvT = nc.dram_tensor("xvT", [KT_E, NQC, 128, 512], BF16, kind="ExternalInput").ap()
    wqT = nc.dram_tensor("wqT", [EMB, DQ], BF16, kind="ExternalInput").ap()
    wkT = nc.dram_tensor("wkT", [EMB, DQ], BF16, kind="ExternalInput").ap()
    wvT = nc.dram_tensor("wvT", [EMB, DQ], BF16, kind="ExternalInput").ap()
    woT = nc.dram_tensor("woT", [DQ, EMB], BF16, kind="ExternalInput").ap()
    bq = nc.dram_tensor("bq", [1, DQ], F32, kind="ExternalInput").ap()
    bk = nc.dram_tensor("bk", [1, DQ], F32, kind="ExternalInput").ap()
    bv = nc.dram_tensor("bv", [1, DQ], F32, kind="ExternalInput").ap()
    out = nc.dram_tensor("out", [S, EMB], BF16, kind="ExternalOutput").ap()

    with ExitStack() as ctx:
        tc = ctx.enter_context(tile.TileContext(nc))
        _mha(ctx, tc, xqT, xkT, xvT, wqT, wkT, wvT, woT, bq, bk, bv, out,
             bench_iters=bench_iters)
    nc.compile()
    return nc


def _chunk_major(x):
    """[S, EMB] fp32 -> bf16 x.T chunked as [KT_E, NQC, 128, 512] (contiguous)."""
    xt = np.asarray(x, np.float32).T.astype(BF16_NP)  # [EMB, S]
    return np.ascontiguousarray(
        xt.reshape(KT_E, 128, NQC, 512).transpose(0, 2, 1, 3)
    )


def _bf(x):
    return np.ascontiguousarray(np.asarray(x, np.float32).astype(BF16_NP))


def make_in_maps(inputs):
    i = {k: np.asarray(v, np.float32) for k, v in inputs.items()}
    xcm = {}
    for nm in ("query", "key", "value"):
        for b in range(B):
            xcm[(nm, b)] = _chunk_major(i[nm][b])
    in_maps = []
    for c in range(NCORES):
        b, g = divmod(c, 4)
        rows = slice(g * DQ, (g + 1) * DQ)
        in_maps.append({
            "xqT": xcm[("query", b)],
            "xkT": xcm[("key", b)],
            "xvT": xcm[("value", b)],
            "wqT": _bf(i["Wq"][rows].T),
            "wkT": _bf(i["Wk"][rows].T),
            "wvT": _bf(i["Wv"][rows].T),
            "woT": _bf(i["Wo"][:, rows].T),
            "bq": np.ascontiguousarray(i["bq"][rows][None, :]),
            "bk": np.ascontiguousarray(i["bk"][rows][None, :]),
            "bv": np.ascontiguousarray(i["bv"][rows][None, :]),
        })
    return in_maps


def kernel(query, key, value, Wq, bq, Wk, bk, Wv, bv, Wo, bo):
    global _NC, LAST_RESULT
    if _NC is None:
        _NC = _build_nc()

    in_maps = make_in_maps({
        "query": query, "key": key, "value": value,
        "Wq": Wq, "bq": bq, "Wk": Wk, "bk": bk,
        "Wv": Wv, "bv": bv, "Wo": Wo, "bo": bo,
    })

    res = bass_utils.run_bass_kernel_spmd(
        _NC, in_maps, core_ids=list(range(NCORES)), trace=TRACE
    )
    LAST_RESULT = res

    out = np.zeros((B, S, EMB), np.float32)
    for c in range(NCORES):
        out[c // 4] += np.asarray(res.results[c]["out"], np.float32)
    out += np.asarray(bo, np.float32)[None, None, :]
    return out
